# revision 1
# baseline (speedup 1.0000x reference)
"""Neural CDE (RK4, 10 steps) Trainium2 Bass/Tile kernel.

Data-parallel over batch: B=1024 split as 128 per core across 8 NeuronCores.
Weights replicated; no collectives.

Per-core math (BS=128 on SBUF partitions):
  z0 = a[:,0] @ W_init + b_init
  per RK4 stage:  dX is one of 21 precomputed vectors (spline derivative at
  the stage's time, which only depends on coeffs, not z):
      hT   = tanh(W1.T-contract zT + b1)           (PE + ACT, h on partitions)
      F    = hT.T @ W2  in 1024-wide chunks        (PE, bf16, f32 PSUM accum)
      k    = segmented-reduce_c(F * rep(dX)) + dX @ b2r.T   (ACT copy, DVE
             mul + reduce, small PE matmul for the b2 term)
  RK4 combine in f32 on DVE. Output out[t] = z_t @ W_out + b_out per step.
"""

import sys
import numpy as np

for _p in ("/opt/trn_rl_repo",):
    if _p not in sys.path:
        sys.path.insert(0, _p)

import ml_dtypes
from contextlib import ExitStack

import concourse.bass as bass
import concourse.bacc as bacc
import concourse.mybir as mybir
import concourse.tile as tile
from concourse.masks import make_identity
from concourse.bass_utils import run_bass_kernel_spmd

B, T, C, H = 1024, 11, 64, 256
NCORES = 8
BS = B // NCORES          # 128
HC = H * C                # 16384
CHUNK = 1024              # F free-dim chunk = 2 matmul windows of 512
NCHUNK = HC // CHUNK      # 16
NW = CHUNK // 512         # windows per chunk

f32 = np.float32
bf16 = ml_dtypes.bfloat16
FP32 = mybir.dt.float32
BF16 = mybir.dt.bfloat16
AO = mybir.AluOpType
AF = mybir.ActivationFunctionType
AX = mybir.AxisListType


def _stage_consts(t_span: np.ndarray):
    """Host-side f32 scalar constants mimicking the reference's fp32 ops."""
    t = np.asarray(t_span, dtype=f32)
    cs = []
    for i in range(T - 1):
        t0 = t[i]
        dt = f32(t[i + 1] - t0)
        tm = f32(t0 + f32(f32(0.5) * dt))
        idx_m = int(np.clip(np.searchsorted(t, tm, side="right") - 1, 0, T - 2))
        fm = f32(tm - t[idx_m])
        cs.append((float(dt), idx_m, float(fm)))
    # final-stage frac for step T-2 (t lands on t_span[-1], idx clamps to T-2)
    fr_last = f32(t[T - 1] - t[T - 2])
    return cs, float(fr_last)


def _build_program(t_span: np.ndarray):
    cs, fr_last = _stage_consts(t_span)

    nc = bacc.Bacc("TRN2", target_bir_lowering=False, debug=False,
                   enable_asserts=False, num_devices=NCORES)

    coeffs_d = nc.dram_tensor("coeffs", [BS, T - 1, 4 * C], FP32, kind="ExternalInput").ap()
    w1_d = nc.dram_tensor("w1", [H, H], BF16, kind="ExternalInput").ap()
    w2_d = nc.dram_tensor("w2", [H, HC], BF16, kind="ExternalInput").ap()
    b1_d = nc.dram_tensor("b1", [H], FP32, kind="ExternalInput").ap()
    b2rt_d = nc.dram_tensor("b2rt", [C, H], BF16, kind="ExternalInput").ap()
    winit_d = nc.dram_tensor("winit", [C, H], BF16, kind="ExternalInput").ap()
    wout_d = nc.dram_tensor("wout", [H, C], FP32, kind="ExternalInput").ap()
    binit_d = nc.dram_tensor("binit", [1, H], FP32, kind="ExternalInput").ap()
    bout_d = nc.dram_tensor("bout", [1, C], FP32, kind="ExternalInput").ap()
    out_d = nc.dram_tensor("out", [BS, T * C], FP32, kind="ExternalOutput").ap()

    with tile.TileContext(nc) as tc, ExitStack() as ctx:
        const = ctx.enter_context(tc.tile_pool(name="const", bufs=1))
        spool = ctx.enter_context(tc.tile_pool(name="stage", bufs=2))
        zpool = ctx.enter_context(tc.tile_pool(name="z", bufs=2))
        kbpool = ctx.enter_context(tc.tile_pool(name="kb", bufs=5))
        fpool = ctx.enter_context(tc.tile_pool(name="fsb", bufs=6))
        gpool = ctx.enter_context(tc.tile_pool(name="gsb", bufs=4))
        pp = ctx.enter_context(tc.tile_pool(name="psmm", bufs=4, space="PSUM"))
        fp = ctx.enter_context(tc.tile_pool(name="psfp", bufs=2, space="PSUM"))

        # ---- resident tensors -------------------------------------------
        coeffs_sb = const.tile([BS, (T - 1) * 4 * C], FP32, tag="coeffs")
        w1_sb = const.tile([128, 2 * H], BF16, tag="w1")
        w2_sb = const.tile([128, 2 * HC], BF16, tag="w2")
        b1_sb = const.tile([128, 2], FP32, tag="b1")
        b2rt_sb = const.tile([C, H], BF16, tag="b2rt")
        winit_sb = const.tile([C, H], BF16, tag="winit")
        wout_sb = const.tile([128, 2 * C], FP32, tag="wout")
        binit_sb = const.tile([1, H], FP32, tag="binit")
        bout_sb = const.tile([1, C], FP32, tag="bout")
        ones1_sb = const.tile([1, 128], FP32, tag="ones1")
        ident = const.tile([128, 128], FP32, tag="ident")
        binit_rep = const.tile([128, H], FP32, tag="binit_rep")
        bout_rep = const.tile([128, C], FP32, tag="bout_rep")
        dxm_sb = const.tile([128, 11 * C], FP32, tag="dxm")      # 10 mids + last-end
        dxT_sb = const.tile([C, 21 * 128], BF16, tag="dxT")
        dxrep_sb = const.tile([128, 21 * CHUNK], BF16, tag="dxrep")
        out_sb = const.tile([BS, T * C], FP32, tag="out_sb")

        nc.sync.dma_start(out=coeffs_sb[:], in_=coeffs_d.rearrange("p i j -> p (i j)"))
        nc.sync.dma_start(out=w1_sb.rearrange("p (k h) -> p k h", k=2),
                          in_=w1_d.rearrange("(k p) h -> p k h", p=128))
        nc.sync.dma_start(out=w2_sb.rearrange("p (k m) -> p k m", k=2),
                          in_=w2_d.rearrange("(k p) m -> p k m", p=128))
        nc.sync.dma_start(out=b1_sb[:], in_=b1_d.rearrange("(k p) -> p k", p=128))
        nc.sync.dma_start(out=b2rt_sb[:], in_=b2rt_d)
        nc.sync.dma_start(out=winit_sb[:], in_=winit_d)
        nc.sync.dma_start(out=wout_sb.rearrange("p (k c) -> p k c", k=2),
                          in_=wout_d.rearrange("(k p) c -> p k c", p=128))
        nc.sync.dma_start(out=binit_sb[:], in_=binit_d)
        nc.sync.dma_start(out=bout_sb[:], in_=bout_d)

        nc.vector.memset(ones1_sb[:], 1.0)
        make_identity(nc, ident[:])

        def cview(i, part):
            """f32 view of coeff column `part` (0=a,1=b,2=2c,3=3d) of interval i."""
            off = i * 4 * C + part * C
            return coeffs_sb[:, off:off + C]

        def dx_f32(s):
            if s < 10:
                return cview(s, 1)
            return dxm_sb[:, (s - 10) * C:(s - 9) * C]

        # ---- dX mid/end vectors (f32) -----------------------------------
        tmp_pool = ctx.enter_context(tc.tile_pool(name="tmp64", bufs=2))
        for i in range(T - 1):
            dt_i, im, fm = cs[i]
            tmp = tmp_pool.tile([128, C], FP32, tag="t64")
            nc.vector.scalar_tensor_tensor(
                out=tmp[:], in0=cview(im, 3), scalar=float(fm), in1=cview(im, 2),
                op0=AO.mult, op1=AO.add)
            nc.vector.scalar_tensor_tensor(
                out=dxm_sb[:, i * C:(i + 1) * C], in0=tmp[:], scalar=float(fm),
                in1=cview(im, 1), op0=AO.mult, op1=AO.add)
        # end-of-grid vector for the very last stage (s == 20)
        tmp = tmp_pool.tile([128, C], FP32, tag="t64")
        nc.vector.scalar_tensor_tensor(
            out=tmp[:], in0=cview(T - 2, 3), scalar=float(fr_last), in1=cview(T - 2, 2),
            op0=AO.mult, op1=AO.add)
        nc.vector.scalar_tensor_tensor(
            out=dxm_sb[:, 10 * C:11 * C], in0=tmp[:], scalar=float(fr_last),
            in1=cview(T - 2, 1), op0=AO.mult, op1=AO.add)

        # ---- dX transposes (bf16) + repeated copies (bf16) ---------------
        for s in range(21):
            src = dx_f32(s)
            ps = pp.tile([128, H], FP32, tag="mm")
            nc.tensor.transpose(ps[0:C, 0:128], src, ident[:])
            nc.scalar.copy(dxT_sb[:, s * 128:(s + 1) * 128], ps[0:C, 0:128])
            for r in range(CHUNK // C):
                nc.scalar.copy(dxrep_sb[:, s * CHUNK + r * C: s * CHUNK + (r + 1) * C], src)

        # ---- replicated biases -------------------------------------------
        ps = pp.tile([128, H], FP32, tag="mm")
        nc.tensor.matmul(ps[:, 0:H], lhsT=ones1_sb[:], rhs=binit_sb[:], start=True, stop=True)
        nc.scalar.copy(binit_rep[:], ps[:, 0:H])
        ps = pp.tile([128, H], FP32, tag="mm")
        nc.tensor.matmul(ps[:, 0:C], lhsT=ones1_sb[:], rhs=bout_sb[:], start=True, stop=True)
        nc.scalar.copy(bout_rep[:], ps[:, 0:C])

        # ---- z0 ----------------------------------------------------------
        ps = pp.tile([128, H], FP32, tag="mm")
        nc.tensor.transpose(ps[0:C, 0:128], cview(0, 0), ident[:])
        x0T_sb = spool.tile([C, 128], BF16, tag="x0T")
        nc.scalar.copy(x0T_sb[:], ps[0:C, 0:128])
        ps = pp.tile([128, H], FP32, tag="mm")
        nc.tensor.matmul(ps[:, 0:H], lhsT=x0T_sb[:], rhs=winit_sb[:], start=True, stop=True)
        z = zpool.tile([BS, H], FP32, tag="z")
        nc.vector.tensor_tensor(out=z[:], in0=ps[:, 0:H], in1=binit_rep[:], op=AO.add)

        # ---- one RK4 stage ----------------------------------------------
        # Returns (ksum, bc_ps, zb, pre): ksum = segmented-reduced F*dX;
        # bc_ps = dX @ b2r.T (PSUM); zb = zbase + alpha*bc (hoisted off the
        # critical tail); pre = pre_add + bc (for the RK4 combine).
        def gstage(zin, s, alpha=None, zbase=None, pre_add=None, emit_out_t=None):
            # transpose zin -> zT (two separate PSUM tiles = different banks, so
            # the copy of half 0 overlaps the transpose of half 1)
            zt_psA = pp.tile([128, 128], FP32, tag="mm")
            zt_psB = pp.tile([128, 128], FP32, tag="mm")
            nc.tensor.transpose(zt_psA[:], zin[:, 0:128], ident[:])
            nc.tensor.transpose(zt_psB[:], zin[:, 128:256], ident[:])
            zTb = spool.tile([128, H], BF16, tag="zTb")
            nc.scalar.copy(zTb[:, 0:128], zt_psA[:])
            nc.scalar.copy(zTb[:, 128:256], zt_psB[:])

            # b2-term: bc = dX @ b2r.T   (PSUM, lives through the stage)
            bc_ps = pp.tile([128, H], FP32, tag="mm")
            nc.tensor.matmul(bc_ps[:], lhsT=dxT_sb[:, s * 128:(s + 1) * 128],
                             rhs=b2rt_sb[:], start=True, stop=True)
            zb = None
            if alpha is not None:
                zb = zpool.tile([BS, H], FP32, tag="zb")
                nc.vector.scalar_tensor_tensor(out=zb[:], in0=bc_ps[:], scalar=float(alpha),
                                               in1=zbase[:], op0=AO.mult, op1=AO.add)
            pre = None
            if pre_add is not None:
                pre = kbpool.tile([BS, H], FP32, tag="pre")
                nc.vector.tensor_tensor(out=pre[:], in0=pre_add[:], in1=bc_ps[:], op=AO.add)

            if emit_out_t is not None:
                t_idx = emit_out_t
                zTf = spool.tile([128, H], FP32, tag="zTf")
                nc.scalar.copy(zTf[:, 0:128], zt_psA[:])
                nc.scalar.copy(zTf[:, 128:256], zt_psB[:])
                ot_ps = pp.tile([128, H], FP32, tag="mm")
                for kc in range(2):
                    nc.tensor.matmul(ot_ps[:, 0:C], lhsT=zTf[:, kc * 128:(kc + 1) * 128],
                                     rhs=wout_sb[:, kc * C:(kc + 1) * C],
                                     start=(kc == 0), stop=(kc == 1))
                nc.vector.tensor_tensor(out=out_sb[:, t_idx * C:(t_idx + 1) * C],
                                        in0=ot_ps[:, 0:C], in1=bout_rep[:], op=AO.add)

            # hT = tanh(W1.T zT + b1)
            ht_ps = pp.tile([128, H], FP32, tag="mm")
            for hck in range(2):
                for kc in range(2):
                    nc.tensor.matmul(
                        ht_ps[:, hck * 128:(hck + 1) * 128],
                        lhsT=w1_sb[:, kc * H + hck * 128: kc * H + (hck + 1) * 128],
                        rhs=zTb[:, kc * 128:(kc + 1) * 128],
                        start=(kc == 0), stop=(kc == 1))
            hT0 = spool.tile([128, 128], BF16, tag="hT0")
            hT1 = spool.tile([128, 128], BF16, tag="hT1")
            for hck, ht_t in enumerate((hT0, hT1)):
                nc.scalar.activation(ht_t[:],
                                     ht_ps[:, hck * 128:(hck + 1) * 128],
                                     AF.Tanh, bias=b1_sb[:, hck:hck + 1], scale=1.0)
            hT_half = (hT0, hT1)

            # F chunks -> scaled -> segment-reduced
            ksum = kbpool.tile([BS, H], FP32, tag="ksum")
            # first two chunks are half-width so the ACT/DVE pipeline primes sooner
            chunks = [(0, 512), (512, 512)] + [(1024 * (j + 1), 1024) for j in range(15)]
            for off, cw in chunks:
                fps = fp.tile([128, cw], FP32, tag="fp")
                for kc in range(2):
                    for w in range(cw // 512):
                        col = kc * HC + off + w * 512
                        nc.tensor.matmul(fps[:, w * 512:(w + 1) * 512],
                                         lhsT=hT_half[kc][:],
                                         rhs=w2_sb[:, col:col + 512],
                                         start=(kc == 0), stop=(kc == 1),
                                         skip_group_check=True)
                fsb = fpool.tile([128, cw], BF16, tag="fsb")
                nc.scalar.copy(fsb[:], fps[:])
                gsb = gpool.tile([128, cw], BF16, tag="gsb")
                nc.vector.tensor_tensor(out=gsb[:], in0=fsb[:],
                                        in1=dxrep_sb[:, s * CHUNK:s * CHUNK + cw],
                                        op=AO.mult)
                nc.vector.tensor_reduce(
                    out=ksum[:, off // C:(off + cw) // C],
                    in_=gsb.rearrange("p (s c) -> p s c", c=C),
                    axis=AX.X, op=AO.add)
            return ksum, bc_ps, zb, pre

        # ---- RK4 time loop ----------------------------------------------
        for i in range(T - 1):
            dt_i, im, fm = cs[i]
            hdt = float(f32(f32(0.5) * f32(dt_i)))
            dt6 = float(f32(f32(dt_i) / f32(6.0)))
            s_m = 10 + i
            s_e = (i + 1) if i < T - 2 else 20

            def kfull(ksum, bc_ps):
                kb = kbpool.tile([BS, H], FP32, tag="kb")
                nc.vector.tensor_tensor(out=kb[:], in0=ksum[:], in1=bc_ps[:], op=AO.add)
                return kb

            k1s, bc1, zb1, _ = gstage(z, i, alpha=hdt, zbase=z, emit_out_t=i)
            zs = zpool.tile([BS, H], FP32, tag="zs")
            for hh in (slice(0, 128), slice(128, 256)):
                nc.vector.scalar_tensor_tensor(out=zs[:, hh], in0=k1s[:, hh], scalar=hdt,
                                               in1=zb1[:, hh], op0=AO.mult, op1=AO.add)
            kb1 = kfull(k1s, bc1)

            k2s, bc2, zb2, _ = gstage(zs, s_m, alpha=hdt, zbase=z)
            zs = zpool.tile([BS, H], FP32, tag="zs")
            for hh in (slice(0, 128), slice(128, 256)):
                nc.vector.scalar_tensor_tensor(out=zs[:, hh], in0=k2s[:, hh], scalar=hdt,
                                               in1=zb2[:, hh], op0=AO.mult, op1=AO.add)
            kb2 = kfull(k2s, bc2)

            k3s, bc3, zb3, _ = gstage(zs, s_m, alpha=float(dt_i), zbase=z)
            zs = zpool.tile([BS, H], FP32, tag="zs")
            for hh in (slice(0, 128), slice(128, 256)):
                nc.vector.scalar_tensor_tensor(out=zs[:, hh], in0=k3s[:, hh], scalar=float(dt_i),
                                               in1=zb3[:, hh], op0=AO.mult, op1=AO.add)
            kb3 = kfull(k3s, bc3)

            # partial RK4 combine (ready before k4's reduces finish)
            acc = kbpool.tile([BS, H], FP32, tag="acc")
            nc.vector.scalar_tensor_tensor(out=acc[:], in0=kb2[:], scalar=2.0, in1=kb1[:],
                                           op0=AO.mult, op1=AO.add)
            acc2 = kbpool.tile([BS, H], FP32, tag="acc2")
            nc.vector.scalar_tensor_tensor(out=acc2[:], in0=kb3[:], scalar=2.0, in1=acc[:],
                                           op0=AO.mult, op1=AO.add)

            k4s, _, _, pre = gstage(zs, s_e, pre_add=acc2)
            acc3 = kbpool.tile([BS, H], FP32, tag="acc3")
            znew = zpool.tile([BS, H], FP32, tag="z")
            for hh in (slice(0, 128), slice(128, 256)):
                nc.vector.tensor_tensor(out=acc3[:, hh], in0=k4s[:, hh], in1=pre[:, hh], op=AO.add)
                nc.vector.scalar_tensor_tensor(out=znew[:, hh], in0=acc3[:, hh], scalar=dt6,
                                               in1=z[:, hh], op0=AO.mult, op1=AO.add)
            z = znew

        # ---- final out row (t = T-1) ------------------------------------
        zt_psA = pp.tile([128, 128], FP32, tag="mm")
        zt_psB = pp.tile([128, 128], FP32, tag="mm")
        nc.tensor.transpose(zt_psA[:], z[:, 0:128], ident[:])
        nc.tensor.transpose(zt_psB[:], z[:, 128:256], ident[:])
        zTf = spool.tile([128, H], FP32, tag="zTf")
        nc.scalar.copy(zTf[:, 0:128], zt_psA[:])
        nc.scalar.copy(zTf[:, 128:256], zt_psB[:])
        ot_ps = pp.tile([128, H], FP32, tag="mm")
        for kc in range(2):
            nc.tensor.matmul(ot_ps[:, 0:C], lhsT=zTf[:, kc * 128:(kc + 1) * 128],
                             rhs=wout_sb[:, kc * C:(kc + 1) * C],
                             start=(kc == 0), stop=(kc == 1))
        nc.vector.tensor_tensor(out=out_sb[:, (T - 1) * C:T * C],
                                in0=ot_ps[:, 0:C], in1=bout_rep[:], op=AO.add)

        nc.sync.dma_start(out=out_d, in_=out_sb[:])

    nc.compile()
    return nc


_CACHE = {}


def _get_program(t_span: np.ndarray):
    key = np.asarray(t_span, dtype=f32).tobytes()
    if key not in _CACHE:
        _CACHE[key] = _build_program(t_span)
    return _CACHE[key]


def _make_in_maps(inputs):
    coeffs = np.ascontiguousarray(inputs["coeffs"], dtype=f32)
    assert coeffs.shape == (B, T - 1, 4 * C)
    shared = {
        "w1": np.ascontiguousarray(inputs["W1"], dtype=f32).astype(bf16),
        "w2": np.ascontiguousarray(inputs["W2"], dtype=f32).astype(bf16),
        "b1": np.ascontiguousarray(inputs["b1"], dtype=f32),
        "b2rt": np.ascontiguousarray(
            np.asarray(inputs["b2"], dtype=f32).reshape(H, C).T).astype(bf16),
        "winit": np.ascontiguousarray(inputs["W_init"], dtype=f32).astype(bf16),
        "wout": np.ascontiguousarray(inputs["W_out"], dtype=f32),
        "binit": np.ascontiguousarray(inputs["b_init"], dtype=f32).reshape(1, H),
        "bout": np.ascontiguousarray(inputs["b_out"], dtype=f32).reshape(1, C),
    }
    in_maps = []
    for c in range(NCORES):
        m = dict(shared)
        m["coeffs"] = coeffs[c * BS:(c + 1) * BS]
        in_maps.append(m)
    return in_maps


def kernel(coeffs, t_span, W_init, b_init, W1, b1, W2, b2, W_out, b_out):
    nc = _get_program(t_span)
    in_maps = _make_in_maps(dict(coeffs=coeffs, W_init=W_init, b_init=b_init,
                                 W1=W1, b1=b1, W2=W2, b2=b2,
                                 W_out=W_out, b_out=b_out))
    res = run_bass_kernel_spmd(nc, in_maps, list(range(NCORES)))
    shards = [res.results[c]["out"].reshape(BS, T, C) for c in range(NCORES)]
    return np.ascontiguousarray(np.concatenate(shards, axis=0), dtype=f32)


if __name__ == "__main__":
    rng = np.random.default_rng(0)
    demo = dict(
        coeffs=(rng.standard_normal((B, T - 1, 4 * C)) * 0.5).astype(f32),
        t_span=(np.arange(T) * 0.05).astype(f32),
        W_init=(rng.standard_normal((C, H)) / 8).astype(f32),
        b_init=(rng.standard_normal((H,)) * 0.01).astype(f32),
        W1=(rng.standard_normal((H, H)) / 16).astype(f32),
        b1=(rng.standard_normal((H,)) * 0.01).astype(f32),
        W2=(rng.standard_normal((H, HC)) / 16).astype(f32),
        b2=(rng.standard_normal((HC,)) * 0.01).astype(f32),
        W_out=(rng.standard_normal((H, C)) / 16).astype(f32),
        b_out=np.zeros((C,), f32),
    )
    out = kernel(**demo)
    print("out", out.shape, out.dtype, float(np.abs(out).max()))



# revision 17
# speedup vs baseline: 1.1093x; 1.1093x over previous
"""Neural CDE (RK4, 10 steps) Trainium2 Bass/Tile kernel.

Data-parallel over batch: B=1024 split as 128 per core across 8 NeuronCores.
Weights replicated; no collectives.

Core trick: k[b,h] = sum_{c,j} (h[b,j] * dX[b,c]) * W2[j, h*C+c].
Instead of materializing F = h @ W2 (16K-wide PSUM intermediate that must be
evacuated, multiplied by dX, and segment-reduced), we build the Khatri-Rao
style stationary operand hcT[(c,j), b] = hT[j,b] * dxT[c,b] with ONE 2x-mode
DVE multiply (dX^T partition-replicated via a broadcast DMA from DRAM), and
accumulate 128 matmuls against a host-repacked W2ch[(c,j), h] directly into a
single [128 b, 256 h] PSUM tile.  The whole per-stage epilogue collapses to
one 256-wide PSUM evacuation.  dX at the 21 stage times depends only on
coeffs/t_span, so dX / dX^T / its replicated DRAM image are host-precomputed
input tensors (input marshaling, same as the W2 repack).
"""

import sys
import numpy as np

for _p in ("/opt/trn_rl_repo",):
    if _p not in sys.path:
        sys.path.insert(0, _p)

import ml_dtypes
from contextlib import ExitStack

import concourse.bass as bass
import concourse.bacc as bacc
import concourse.mybir as mybir
import concourse.tile as tile
from concourse.masks import make_identity
from concourse.bass_utils import run_bass_kernel_spmd

B, T, C, H = 1024, 11, 64, 256
NCORES = 8
BS = B // NCORES          # 128
HC = H * C                # 16384
NS = 21                   # distinct dX stage vectors
KT = 128                  # k-matmul passes per stage (c,kc)
DXW = C * BS              # 8192: flattened dX^T per stage

f32 = np.float32
bf16 = ml_dtypes.bfloat16
FP32 = mybir.dt.float32
BF16 = mybir.dt.bfloat16
AO = mybir.AluOpType
AF = mybir.ActivationFunctionType

# hcT build: 8 blocks of 8 c-values; blocks run on DVE except these on Pool
POOL_BLOCKS = ()
DEBUG_TAPS = False


def _stage_consts(t_span: np.ndarray):
    """Host-side f32 scalar constants mimicking the reference's fp32 ops."""
    t = np.asarray(t_span, dtype=f32)
    cs = []
    for i in range(T - 1):
        t0 = t[i]
        dt = f32(t[i + 1] - t0)
        tm = f32(t0 + f32(f32(0.5) * dt))
        idx_m = int(np.clip(np.searchsorted(t, tm, side="right") - 1, 0, T - 2))
        fm = f32(tm - t[idx_m])
        cs.append((float(dt), idx_m, float(fm)))
    fr_last = f32(t[T - 1] - t[T - 2])
    return cs, float(fr_last)


def _host_dx(coeffs_core: np.ndarray, t_span: np.ndarray):
    """The 21 spline-derivative vectors for one core's batch slice, f32.

    Stages 0..9: dX at t_i (= b coeff of interval i).  Stages 10..19: dX at
    the RK4 midpoints.  Stage 20: dX at t_{T-1} (interval T-2, frac = dt).
    Mirrors reference._spline_deriv in f32.
    """
    cs, fr_last = _stage_consts(t_span)
    a, b, two_c, three_d = np.split(coeffs_core.astype(f32), 4, axis=-1)
    dxs = []
    for s in range(10):
        dxs.append(b[:, s])
    for i in range(T - 1):
        _, im, fm = cs[i]
        fm = f32(fm)
        dxs.append(b[:, im] + (two_c[:, im] + three_d[:, im] * fm) * fm)
    im, fm = T - 2, f32(fr_last)
    dxs.append(b[:, im] + (two_c[:, im] + three_d[:, im] * fm) * fm)
    assert len(dxs) == NS
    return np.stack(dxs, 0).astype(f32)  # (21, BS, C)


def _build_program(t_span: np.ndarray):
    cs, _ = _stage_consts(t_span)

    nc = bacc.Bacc("TRN2", target_bir_lowering=False, debug=False,
                   enable_asserts=False, num_devices=NCORES)

    x0_d = nc.dram_tensor("x0", [BS, C], FP32, kind="ExternalInput").ap()
    dxpt_d = nc.dram_tensor("dxpt", [NS, DXW], BF16, kind="ExternalInput").ap()
    dxT_d = nc.dram_tensor("dxt", [C, NS * BS], BF16, kind="ExternalInput").ap()
    w1_d = nc.dram_tensor("w1", [H, H], BF16, kind="ExternalInput").ap()
    w2ch_d = nc.dram_tensor("w2ch", [KT, 128, H], BF16, kind="ExternalInput").ap()
    b1_d = nc.dram_tensor("b1", [H], FP32, kind="ExternalInput").ap()
    b2rt_d = nc.dram_tensor("b2rt", [C, H], BF16, kind="ExternalInput").ap()
    winit_d = nc.dram_tensor("winit", [C, H], BF16, kind="ExternalInput").ap()
    wout_d = nc.dram_tensor("wout", [H, C], BF16, kind="ExternalInput").ap()
    binit_d = nc.dram_tensor("binit", [1, H], FP32, kind="ExternalInput").ap()
    bout_d = nc.dram_tensor("bout", [1, C], FP32, kind="ExternalInput").ap()
    out_d = nc.dram_tensor("out", [BS, T * C], FP32, kind="ExternalOutput").ap()
    if DEBUG_TAPS:
        dbg_dxpt_d = nc.dram_tensor("dbg_dxpt", [128, DXW], BF16, kind="ExternalOutput").ap()
        dbg_hct_d = nc.dram_tensor("dbg_hct", [128, C * 2 * 128], BF16, kind="ExternalOutput").ap()
        dbg_ksum_d = nc.dram_tensor("dbg_ksum", [BS, H], FP32, kind="ExternalOutput").ap()
        dbg_ht_d = nc.dram_tensor("dbg_ht", [128, H], BF16, kind="ExternalOutput").ap()

    with tile.TileContext(nc) as tc, ExitStack() as ctx:
        const = ctx.enter_context(tc.tile_pool(name="const", bufs=1))
        spool = ctx.enter_context(tc.tile_pool(name="stage", bufs=2))
        zpool = ctx.enter_context(tc.tile_pool(name="z", bufs=2))
        kbpool = ctx.enter_context(tc.tile_pool(name="kb", bufs=3))
        hpool = ctx.enter_context(tc.tile_pool(name="hct", bufs=1))
        dxp = ctx.enter_context(tc.tile_pool(name="dxp", bufs=2))
        pp = ctx.enter_context(tc.tile_pool(name="psmm", bufs=4, space="PSUM"))
        kp = ctx.enter_context(tc.tile_pool(name="pskk", bufs=2, space="PSUM"))

        # ---- resident tensors -------------------------------------------
        x0_sb = const.tile([BS, C], FP32, tag="x0")
        w1_sb = const.tile([128, 2 * H], BF16, tag="w1")
        w2ch_sb = const.tile([128, KT * H], BF16, tag="w2ch")
        b1_sb = const.tile([128, 2], FP32, tag="b1")
        b2rt_sb = const.tile([C, H], BF16, tag="b2rt")
        winit_sb = const.tile([C, H], BF16, tag="winit")
        wout_sb = const.tile([128, 2 * C], BF16, tag="wout")
        binit_sb = const.tile([1, H], FP32, tag="binit")
        bout_sb = const.tile([1, C], FP32, tag="bout")
        ones1_sb = const.tile([1, 128], FP32, tag="ones1")
        ident = const.tile([128, 128], FP32, tag="ident")
        binit_rep = const.tile([128, H], FP32, tag="binit_rep")
        bout_rep = const.tile([128, C], FP32, tag="bout_rep")
        dxT_sb = const.tile([C, NS * BS], BF16, tag="dxT")
        out_sb = const.tile([BS, T * C], FP32, tag="out_sb")

        nc.sync.dma_start(out=x0_sb[:], in_=x0_d)
        nc.sync.dma_start(out=w1_sb.rearrange("p (k h) -> p k h", k=2),
                          in_=w1_d.rearrange("(k p) h -> p k h", p=128))
        # split w2ch DMA so early k-passes' weights land first
        w2v = w2ch_sb.rearrange("p (k h) -> p k h", k=KT)
        NW2 = 8
        for i in range(NW2):
            sl = slice(i * (KT // NW2), (i + 1) * (KT // NW2))
            nc.sync.dma_start(out=w2v[:, sl, :],
                              in_=w2ch_d.rearrange("k p h -> p k h")[:, sl, :])
        nc.sync.dma_start(out=b1_sb[:], in_=b1_d.rearrange("(k p) -> p k", p=128))
        nc.sync.dma_start(out=b2rt_sb[:], in_=b2rt_d)
        nc.sync.dma_start(out=winit_sb[:], in_=winit_d)
        nc.sync.dma_start(out=wout_sb.rearrange("p (k c) -> p k c", k=2),
                          in_=wout_d.rearrange("(k p) c -> p k c", p=128))
        nc.sync.dma_start(out=binit_sb[:], in_=binit_d)
        nc.sync.dma_start(out=bout_sb[:], in_=bout_d)
        nc.sync.dma_start(out=dxT_sb[:], in_=dxT_d)

        nc.vector.memset(ones1_sb[:], 1.0)
        make_identity(nc, ident[:])

        # ---- replicated biases -------------------------------------------
        ps = pp.tile([128, H], FP32, tag="mm")
        nc.tensor.matmul(ps[:, 0:H], lhsT=ones1_sb[:], rhs=binit_sb[:], start=True, stop=True)
        nc.scalar.copy(binit_rep[:], ps[:, 0:H])
        ps = pp.tile([128, H], FP32, tag="mm")
        nc.tensor.matmul(ps[:, 0:C], lhsT=ones1_sb[:], rhs=bout_sb[:], start=True, stop=True)
        nc.scalar.copy(bout_rep[:], ps[:, 0:C])

        # ---- z0 ----------------------------------------------------------
        ps = pp.tile([128, H], FP32, tag="mm")
        nc.tensor.transpose(ps[0:C, 0:128], x0_sb[:], ident[:])
        x0T_sb = spool.tile([C, 128], BF16, tag="x0T")
        nc.scalar.copy(x0T_sb[:], ps[0:C, 0:128])
        ps = pp.tile([128, H], FP32, tag="mm")
        nc.tensor.matmul(ps[:, 0:H], lhsT=x0T_sb[:], rhs=winit_sb[:], start=True, stop=True)
        z = zpool.tile([BS, H], FP32, tag="z")
        nc.vector.tensor_tensor(out=z[:], in0=ps[:, 0:H], in1=binit_rep[:], op=AO.add)

        # dX^T replicated tiles, prefetched one stage ahead ---------------
        def fetch_dxpt(s):
            t = dxp.tile([128, DXW], BF16, tag="dxpt")
            nc.sync.dma_start(out=t[:],
                              in_=dxpt_d[s:s + 1, :].broadcast_to([128, DXW]))
            return t

        # ---- one RK4 stage ----------------------------------------------
        def gstage(zin, s, dxpt, alpha=None, zbase=None, pre_add=None,
                   emit_out_t=None):
            # transpose zin -> zT (two PSUM tiles = different banks)
            zt_psA = pp.tile([128, 128], FP32, tag="mm")
            zt_psB = pp.tile([128, 128], FP32, tag="mm")
            nc.tensor.transpose(zt_psA[:], zin[:, 0:128], ident[:])
            nc.tensor.transpose(zt_psB[:], zin[:, 128:256], ident[:])
            zTb = spool.tile([128, H], BF16, tag="zTb")
            nc.scalar.copy(zTb[:, 0:128], zt_psA[:])
            nc.scalar.copy(zTb[:, 128:256], zt_psB[:])

            # b2-term: bc = dX @ b2r.T   (PSUM, lives through the stage)
            bc_ps = pp.tile([128, H], FP32, tag="mm")
            nc.tensor.matmul(bc_ps[:], lhsT=dxT_sb[:, s * 128:(s + 1) * 128],
                             rhs=b2rt_sb[:], start=True, stop=True)
            zb = None
            if alpha is not None:
                zb = zpool.tile([BS, H], FP32, tag="zb")
                nc.vector.scalar_tensor_tensor(out=zb[:], in0=bc_ps[:], scalar=float(alpha),
                                               in1=zbase[:], op0=AO.mult, op1=AO.add)
            pre = None
            if pre_add is not None:
                pre = kbpool.tile([BS, H], FP32, tag="pre")
                nc.vector.tensor_tensor(out=pre[:], in0=pre_add[:], in1=bc_ps[:], op=AO.add)

            if emit_out_t is not None:
                t_idx = emit_out_t
                ot_ps = pp.tile([128, H], FP32, tag="mm")
                for kc in range(2):
                    nc.tensor.matmul(ot_ps[:, 0:C], lhsT=zTb[:, kc * 128:(kc + 1) * 128],
                                     rhs=wout_sb[:, kc * C:(kc + 1) * C],
                                     start=(kc == 0), stop=(kc == 1))
                nc.vector.tensor_tensor(out=out_sb[:, t_idx * C:(t_idx + 1) * C],
                                        in0=ot_ps[:, 0:C], in1=bout_rep[:], op=AO.add)

            # hT = tanh(W1.T zT + b1)
            ht_ps = pp.tile([128, H], FP32, tag="mm")
            for hck in range(2):
                for kc in range(2):
                    nc.tensor.matmul(
                        ht_ps[:, hck * 128:(hck + 1) * 128],
                        lhsT=w1_sb[:, kc * H + hck * 128: kc * H + (hck + 1) * 128],
                        rhs=zTb[:, kc * 128:(kc + 1) * 128],
                        start=(kc == 0), stop=(kc == 1),
                        skip_group_check=True)
            hT = spool.tile([128, H], BF16, tag="hT")
            for hck in range(2):
                nc.scalar.activation(hT[:, hck * 128:(hck + 1) * 128],
                                     ht_ps[:, hck * 128:(hck + 1) * 128],
                                     AF.Tanh, bias=b1_sb[:, hck:hck + 1], scale=1.0)

            # hcT[j', (c,kc,b)] = hT[(kc,j'), b] * dxT[c, b]
            hcT = hpool.tile([128, C * 2 * 128], BF16, tag="hcT")  # [128, 16384]
            hview = hT.rearrange("p (k b) -> p k b", k=2)
            dxv = dxpt.rearrange("p (c b) -> p c b", c=C)
            CB = C // 8  # c-values per build block
            for blk in range(8):
                csl = slice(blk * CB, (blk + 1) * CB)
                ov = hcT[:, blk * CB * 2 * 128:(blk + 1) * CB * 2 * 128] \
                    .rearrange("p (c k b) -> p c k b", c=CB, k=2)
                i1_3d = dxv[:, csl, :]                      # [128, CB, 128]
                for k in range(2):
                    i0 = hview[:, k:k + 1, :].broadcast_to([128, CB, 128])
                    if blk in POOL_BLOCKS:
                        nc.vector.scalar_tensor_tensor(
                            out=ov[:, :, k, :], in0=i0, scalar=1.0,
                            in1=i1_3d, op0=AO.mult, op1=AO.mult)
                    else:
                        nc.vector.tensor_tensor(out=ov[:, :, k, :], in0=i0,
                                                in1=i1_3d, op=AO.mult)

            # k = sum_kt hcT_kt.T @ W2ch_kt   (single PSUM accumulation)
            k_ps = kp.tile([128, H], FP32, tag="kk")
            for kt in range(KT):
                nc.tensor.matmul(k_ps[:],
                                 lhsT=hcT[:, kt * 128:(kt + 1) * 128],
                                 rhs=w2ch_sb[:, kt * H:(kt + 1) * H],
                                 start=(kt == 0), stop=(kt == KT - 1))
            ksum = kbpool.tile([BS, H], FP32, tag="ksum")
            nc.scalar.copy(ksum[:], k_ps[:])
            if DEBUG_TAPS and s == 0:
                nc.sync.dma_start(out=dbg_dxpt_d, in_=dxpt[:])
                nc.sync.dma_start(out=dbg_hct_d, in_=hcT[:])
                nc.sync.dma_start(out=dbg_ksum_d, in_=ksum[:])
                nc.sync.dma_start(out=dbg_ht_d, in_=hT[:])
            return ksum, bc_ps, zb, pre

        # ---- RK4 time loop ----------------------------------------------
        dx_next = fetch_dxpt(0)
        for i in range(T - 1):
            dt_i, im, fm = cs[i]
            hdt = float(f32(f32(0.5) * f32(dt_i)))
            dt6 = float(f32(f32(dt_i) / f32(6.0)))
            s_m = 10 + i
            s_e = (i + 1) if i < T - 2 else 20

            def kfull(ksum, bc_ps):
                kb = kbpool.tile([BS, H], FP32, tag="kb")
                nc.vector.tensor_tensor(out=kb[:], in0=ksum[:], in1=bc_ps[:], op=AO.add)
                return kb

            dx1, dx_next = dx_next, fetch_dxpt(s_m)
            k1s, bc1, zb1, _ = gstage(z, i, dx1, alpha=hdt, zbase=z, emit_out_t=i)
            zs = zpool.tile([BS, H], FP32, tag="zs")
            for hh in (slice(0, 128), slice(128, 256)):
                nc.vector.scalar_tensor_tensor(out=zs[:, hh], in0=k1s[:, hh], scalar=hdt,
                                               in1=zb1[:, hh], op0=AO.mult, op1=AO.add)
            kb1 = kfull(k1s, bc1)

            dxm_t = dx_next
            dx_next = fetch_dxpt(s_e)
            k2s, bc2, zb2, _ = gstage(zs, s_m, dxm_t, alpha=hdt, zbase=z)
            zs = zpool.tile([BS, H], FP32, tag="zs")
            for hh in (slice(0, 128), slice(128, 256)):
                nc.vector.scalar_tensor_tensor(out=zs[:, hh], in0=k2s[:, hh], scalar=hdt,
                                               in1=zb2[:, hh], op0=AO.mult, op1=AO.add)
            kb2 = kfull(k2s, bc2)

            k3s, bc3, zb3, _ = gstage(zs, s_m, dxm_t, alpha=float(dt_i), zbase=z)
            zs = zpool.tile([BS, H], FP32, tag="zs")
            for hh in (slice(0, 128), slice(128, 256)):
                nc.vector.scalar_tensor_tensor(out=zs[:, hh], in0=k3s[:, hh], scalar=float(dt_i),
                                               in1=zb3[:, hh], op0=AO.mult, op1=AO.add)
            kb3 = kfull(k3s, bc3)

            # partial RK4 combine (ready before k4 finishes)
            acc = kbpool.tile([BS, H], FP32, tag="acc")
            nc.vector.scalar_tensor_tensor(out=acc[:], in0=kb2[:], scalar=2.0, in1=kb1[:],
                                           op0=AO.mult, op1=AO.add)
            acc2 = kbpool.tile([BS, H], FP32, tag="acc2")
            nc.vector.scalar_tensor_tensor(out=acc2[:], in0=kb3[:], scalar=2.0, in1=acc[:],
                                           op0=AO.mult, op1=AO.add)

            # k4's stage (s_e) is also the next step's k1 stage: reuse the tile
            dx4 = dx_next
            dx_next = dx4
            k4s, _, _, pre = gstage(zs, s_e, dx4, pre_add=acc2)
            acc3 = kbpool.tile([BS, H], FP32, tag="acc3")
            znew = zpool.tile([BS, H], FP32, tag="z")
            for hh in (slice(0, 128), slice(128, 256)):
                nc.vector.scalar_tensor_tensor(out=acc3[:, hh], in0=k4s[:, hh], scalar=1.0,
                                               in1=pre[:, hh], op0=AO.mult, op1=AO.add)
                nc.vector.scalar_tensor_tensor(out=znew[:, hh], in0=acc3[:, hh], scalar=dt6,
                                               in1=z[:, hh], op0=AO.mult, op1=AO.add)
            z = znew

        # ---- final out row (t = T-1) ------------------------------------
        zt_psA = pp.tile([128, 128], FP32, tag="mm")
        zt_psB = pp.tile([128, 128], FP32, tag="mm")
        nc.tensor.transpose(zt_psA[:], z[:, 0:128], ident[:])
        nc.tensor.transpose(zt_psB[:], z[:, 128:256], ident[:])
        zTf = spool.tile([128, H], BF16, tag="zTb")
        nc.scalar.copy(zTf[:, 0:128], zt_psA[:])
        nc.scalar.copy(zTf[:, 128:256], zt_psB[:])
        ot_ps = pp.tile([128, H], FP32, tag="mm")
        for kc in range(2):
            nc.tensor.matmul(ot_ps[:, 0:C], lhsT=zTf[:, kc * 128:(kc + 1) * 128],
                             rhs=wout_sb[:, kc * C:(kc + 1) * C],
                             start=(kc == 0), stop=(kc == 1))
        nc.vector.tensor_tensor(out=out_sb[:, (T - 1) * C:T * C],
                                in0=ot_ps[:, 0:C], in1=bout_rep[:], op=AO.add)

        nc.sync.dma_start(out=out_d, in_=out_sb[:])

    nc.compile()
    return nc


_CACHE = {}


def _get_program(t_span: np.ndarray):
    key = np.asarray(t_span, dtype=f32).tobytes()
    if key not in _CACHE:
        _CACHE[key] = _build_program(t_span)
    return _CACHE[key]


def _make_in_maps(inputs):
    coeffs = np.ascontiguousarray(inputs["coeffs"], dtype=f32)
    t_span = np.asarray(inputs["t_span"], dtype=f32)
    assert coeffs.shape == (B, T - 1, 4 * C)
    W2 = np.ascontiguousarray(inputs["W2"], dtype=f32)  # (H, HC)
    # W2ch[(c*H + j), h] = W2[j, h*C + c]  ->  (KT, 128, H)
    w2r = W2.reshape(H, H, C)                      # (j, h, c)
    w2ch = np.transpose(w2r, (2, 0, 1)).reshape(C * H, H)  # ((c,j), h)
    w2ch = np.ascontiguousarray(w2ch.reshape(KT, 128, H)).astype(bf16)
    shared = {
        "w1": np.ascontiguousarray(inputs["W1"], dtype=f32).astype(bf16),
        "w2ch": w2ch,
        "b1": np.ascontiguousarray(inputs["b1"], dtype=f32),
        "b2rt": np.ascontiguousarray(
            np.asarray(inputs["b2"], dtype=f32).reshape(H, C).T).astype(bf16),
        "winit": np.ascontiguousarray(inputs["W_init"], dtype=f32).astype(bf16),
        "wout": np.ascontiguousarray(inputs["W_out"], dtype=f32).astype(bf16),
        "binit": np.ascontiguousarray(inputs["b_init"], dtype=f32).reshape(1, H),
        "bout": np.ascontiguousarray(inputs["b_out"], dtype=f32).reshape(1, C),
    }
    in_maps = []
    for c in range(NCORES):
        m = dict(shared)
        cc = coeffs[c * BS:(c + 1) * BS]
        dx = _host_dx(cc, t_span)                  # (21, BS, C) f32
        dxT = np.transpose(dx, (2, 0, 1))          # (C, 21, BS)
        m["x0"] = np.ascontiguousarray(cc[:, 0, 0:C])
        m["dxpt"] = np.ascontiguousarray(
            dxT.transpose(1, 0, 2).reshape(NS, DXW)).astype(bf16)
        m["dxt"] = np.ascontiguousarray(dxT.reshape(C, NS * BS)).astype(bf16)
        in_maps.append(m)
    return in_maps


def kernel(coeffs, t_span, W_init, b_init, W1, b1, W2, b2, W_out, b_out):
    nc = _get_program(t_span)
    in_maps = _make_in_maps(dict(coeffs=coeffs, t_span=t_span, W_init=W_init,
                                 b_init=b_init, W1=W1, b1=b1, W2=W2, b2=b2,
                                 W_out=W_out, b_out=b_out))
    res = run_bass_kernel_spmd(nc, in_maps, list(range(NCORES)))
    shards = [res.results[c]["out"].reshape(BS, T, C) for c in range(NCORES)]
    return np.ascontiguousarray(np.concatenate(shards, axis=0), dtype=f32)


if __name__ == "__main__":
    rng = np.random.default_rng(0)
    demo = dict(
        coeffs=(rng.standard_normal((B, T - 1, 4 * C)) * 0.5).astype(f32),
        t_span=(np.arange(T) * 0.05).astype(f32),
        W_init=(rng.standard_normal((C, H)) / 8).astype(f32),
        b_init=(rng.standard_normal((H,)) * 0.01).astype(f32),
        W1=(rng.standard_normal((H, H)) / 16).astype(f32),
        b1=(rng.standard_normal((H,)) * 0.01).astype(f32),
        W2=(rng.standard_normal((H, HC)) / 16).astype(f32),
        b2=(rng.standard_normal((HC,)) * 0.01).astype(f32),
        W_out=(rng.standard_normal((H, C)) / 16).astype(f32),
        b_out=np.zeros((C,), f32),
    )
    out = kernel(**demo)
    print("out", out.shape, out.dtype, float(np.abs(out).max()))
